# revision 40
# baseline (speedup 1.0000x reference)
"""Trainium2 Bass kernel for nn_AttentionPoolingTemporalEncoder.

Strategy (data-parallel over batch, 8 cores, 4 batch rows each):
  device:  h' = relu(x @ (64*Wp))          (fp8 DoubleRow matmuls, 2x rate)
           scores = h' @ ((Wk @ qh)/(64*sqrt(D)))  (bk shifts cancel in softmax)
           p = exp(scores + maskbias)      (no running max; scores are O(5))
           U[h,:] = sum_s p[s,h] * h'[s,:] ; Z[h] = sum_s p[s,h]
  host:    pooled = (U/(64*Z)) @ Wv (+bv) per head; @Wo+bo; @W2+b2; LayerNorm.

v5: the U matmuls (M=8, N=512) are col-tiled — the 4 tiles of each
transpose batch run concurrently in distinct 32-column PE groups
(tile_position=(0,32*tt)), quadrupling U throughput; the host sums the
4 partition groups. The vector engine runs ONLY the relu/cast ops:
every transpose waits on the single per-engine semaphore of its
producers, so any other op in that queue (exp, copies) transitively
couples the transpose to tensor-engine tail progress and stalls the
pipeline — exp and the U/Z drains therefore live on the scalar
engine, DMAs and memsets on gpsimd. x is loaded in 1MB chunks (small
first chunks on the HWDGE scalar queue before any compute queues up);
attention tails trail the projection by 3 transpose batches.
"""

import sys
import threading

import numpy as np

sys.path.insert(0, "/opt/trn_rl_repo")

from contextlib import ExitStack

import concourse.tile as tile
from concourse import bacc, mybir
from concourse.bass_utils import run_bass_kernel_spmd


def _ensure_axon_ntff_hook_module():
    """Some images lack ``antenv.axon_hooks``; concourse imports it
    unconditionally when tracing is requested (e.g. via BASS_TRACE).
    Provide a minimal stand-in so that path degrades to no-trace
    instead of crashing."""
    try:
        from antenv import axon_hooks  # noqa: F401

        return
    except ImportError:
        pass
    import types

    mod = types.ModuleType("antenv.axon_hooks")
    mod._hook = None

    def set_axon_ntff_profile_hook(h):
        mod._hook = h

    def get_axon_ntff_profile_hook():
        return mod._hook

    mod.set_axon_ntff_profile_hook = set_axon_ntff_profile_hook
    mod.get_axon_ntff_profile_hook = get_axon_ntff_profile_hook
    sys.modules["antenv.axon_hooks"] = mod
    try:
        import antenv

        antenv.axon_hooks = mod
    except ImportError:
        pass


_ensure_axon_ntff_hook_module()

# Problem sizes (hardcoded per spec)
B, S, IN_DIM, E, H = 32, 4096, 1024, 512, 8
D = E // H
NCORES = 8
P = 128
WP_SCALE = 64.0  # Wp pre-scaled into fp8's sweet spot; relu is homogeneous
TB = 4           # s-tiles per transpose batch / U col-tile group

_nc_cache = {}
_nc_lock = threading.Lock()


def build_nc(BL=B // NCORES, S_=S, I_=IN_DIM, has_bp=False):
    """Build + compile the per-core Bass program."""
    key = (BL, S_, I_, has_bp)
    with _nc_lock:
        if key in _nc_cache:
            return _nc_cache[key]

    C4 = I_ // 256      # 256-deep DoubleRow contraction chunks
    EC = E // P         # embed-dim chunks
    S_TILES = S_ // P   # s-tiles per batch row

    # x chunk list: (row, s_start, s_len). Small first chunks get the
    # pipeline started early; 1024-s (1MB) chunks keep SDMA ring backlog
    # short so transposes are never stuck behind a large x transfer.
    chunks = []
    for bb in range(BL):
        s0 = 0
        first = bb == 0
        while s0 < S_:
            sl = 512 if (first and s0 < 1024) else 1024
            sl = min(sl, S_ - s0)
            chunks.append((bb, s0, sl))
            s0 += sl

    f32 = mybir.dt.float32
    bf16 = mybir.dt.bfloat16
    fp8 = mybir.dt.float8e4
    EXP = mybir.ActivationFunctionType.Exp
    DR = mybir.MatmulPerfMode.DoubleRow

    nc = bacc.Bacc(
        "TRN2",
        target_bir_lowering=False,
        debug=False,
        enable_asserts=False,
        num_devices=NCORES,
    )

    xt = nc.dram_tensor("xt", [BL, P, C4, 2, S_], fp8, kind="ExternalInput").ap()
    wp = nc.dram_tensor("wp", [P, C4, 2, E], fp8, kind="ExternalInput").ap()
    wkq = nc.dram_tensor("wkq", [EC, P, H], bf16, kind="ExternalInput").ap()
    mb = nc.dram_tensor("mb", [BL, P, S_TILES], f32, kind="ExternalInput").ap()
    if has_bp:
        bp_d = nc.dram_tensor("bp", [1, E], bf16, kind="ExternalInput").ap()
    # col-tiled U: partition group 32*j holds the partial sums of
    # s-tiles with (tile_in_batch == j); host adds the 4 groups.
    u_out = nc.dram_tensor("u_out", [BL, P, E], f32, kind="ExternalOutput").ap()
    z_out = nc.dram_tensor("z_out", [BL, TB * H, 1], f32, kind="ExternalOutput").ap()

    with tile.TileContext(nc) as tc, ExitStack() as ctx:
        const = ctx.enter_context(tc.tile_pool(name="const", bufs=1))
        xp = ctx.enter_context(tc.tile_pool(name="xp", bufs=6))
        hp = ctx.enter_context(tc.tile_pool(name="hp", bufs=7))
        htp = ctx.enter_context(tc.tile_pool(name="htp", bufs=7))
        pp = ctx.enter_context(tc.tile_pool(name="pp", bufs=6))
        mbp = ctx.enter_context(tc.tile_pool(name="mbp", bufs=2))
        uzp = ctx.enter_context(tc.tile_pool(name="uzp", bufs=2))
        ps_h = ctx.enter_context(tc.tile_pool(name="ps_h", bufs=3, space="PSUM"))
        ps_s = ctx.enter_context(tc.tile_pool(name="ps_s", bufs=2, space="PSUM"))
        ps_u = ctx.enter_context(tc.tile_pool(name="ps_u", bufs=2, space="PSUM"))
        ps_z = ctx.enter_context(tc.tile_pool(name="ps_z", bufs=1, space="PSUM"))

        # Resident constants (sync queue, ahead of the transposes)
        wp_sb = const.tile([P, C4, 2, E], fp8)
        nc.sync.dma_start(wp_sb[:], wp[:])
        wkq_sb = const.tile([P, EC, H], bf16)
        nc.sync.dma_start(wkq_sb[:], wkq.rearrange("c p h -> p c h"))
        ones_t = const.tile([P, 2], bf16)
        nc.gpsimd.memset(ones_t[:], 1.0)
        if has_bp:
            ones_row = const.tile([1, P], bf16)
            nc.gpsimd.memset(ones_row[:], 1.0)
            bp_sb = const.tile([1, E], bf16)
            nc.sync.dma_start(bp_sb[:], bp_d[:])

        # x chunks on the scalar HWDGE queue. The issue distance (3) is
        # kept well below the buffer count (6) so each dma_start's
        # buffer-reuse wait refers to a chunk finished long ago — the DMA
        # fires the moment the queue reaches it and the chunk lands well
        # before the sync queue hits Tile's completion-observer for it
        # (those observers are sequenced between the transposes).
        def load_chunk(idx):
            bb, s0, sl = chunks[idx]
            xt_c = xp.tile([P, C4, 2, sl], fp8, tag="xchunk")
            # scalar HWDGE: the queue's natural (exp-paced) progress rate
            # spreads the x transfers out so they never flood the shared
            # SDMA rings and starve the transposes
            nc.scalar.dma_start(xt_c[:], xt[bb, :, :, :, s0 : s0 + sl])
            return xt_c

        # shallow initial prefetch (ring-flood at kernel start delays the
        # first transposes' completion-observers); ramps to distance 3
        bufq = [load_chunk(i) for i in range(min(2, len(chunks)))]
        next_load = len(bufq)

        mb_tiles = {}

        def load_mb(bb):
            mb_t = mbp.tile([P, S_TILES], f32)
            nc.gpsimd.dma_start(mb_t[:], mb[bb])
            mb_tiles[bb] = mb_t

        load_mb(0)
        if BL > 1:
            load_mb(1)

        row_state = {}  # b -> (u_ps, z_ps)
        COPY = mybir.ActivationFunctionType.Copy
        uz_queue = []     # stage-B work: (b, t0, h_se_b, p_b)
        drain_queue = []  # row drains, delayed >= 1 batch so the scalar
                          # copies never wait on in-flight U matmuls

        def emit_drains():
            while drain_queue:
                b_, u_ps, z_ps = drain_queue.pop(0)
                u_sb = uzp.tile([P, E], f32, tag="u_sb")
                z_sb = uzp.tile([TB * H, 1], f32, tag="z_sb")
                nc.scalar.activation(u_sb[:], u_ps[:], COPY)
                nc.scalar.activation(z_sb[:], z_ps[:, 0:1], COPY)
                nc.scalar.dma_start(u_out[b_], u_sb[:])
                nc.scalar.dma_start(z_out[b_], z_sb[:])

        def emit_uz(work):
            """Stage B: U/Z matmuls, one transpose batch behind stage A
            so the exps have a full batch period to complete."""
            b_, t0_, h_se_b, p_b = work
            u_ps, z_ps = row_state[b_]
            # 4 U matmuls run concurrently in distinct 32-col PE groups
            for tt in range(TB):
                t_ = t0_ + tt
                nc.tensor.matmul(
                    u_ps[32 * tt : 32 * tt + H, :],
                    p_b[:, tt, :],
                    h_se_b[:, tt, :],
                    start=(t_ < TB),
                    stop=(t_ >= S_TILES - TB),
                    skip_group_check=True,
                    tile_position=(0, 32 * tt),
                )
            # Z partials for all 4 tiles in ONE M=32 matmul (host sums
            # the 4 head-groups, same as for U)
            nc.tensor.matmul(
                z_ps[:],
                p_b[:],
                ones_t[:],
                start=(t0_ == 0),
                stop=(t0_ + TB == S_TILES),
                skip_group_check=True,
            )
            if t0_ + TB == S_TILES:
                drain_queue.append((b_, u_ps, z_ps))
                del row_state[b_]

        def emit_tails(pend):
            """Stage A for one transpose batch: scores + exp; then stage
            B (U/Z) for the previous batch and any due row drains."""
            b_, t0_, h_se_b, ht_b = pend
            emit_drains()
            if b_ not in row_state:
                u_ps = ps_u.tile([P, E], f32)
                z_ps = ps_z.tile([TB * H, 2], f32)
                row_state[b_] = (u_ps, z_ps)
            mb_t = mb_tiles[b_]
            p_b = pp.tile([P, TB, H], bf16)
            for tt in range(TB):
                t_ = t0_ + tt
                # scores[s,h] = sum_e h[s,e] wkq[e,h]
                sc_ps = ps_s.tile([P, H], f32)
                for ec in range(EC):
                    nc.tensor.matmul(
                        sc_ps[:],
                        ht_b[:, tt * EC + ec, :],
                        wkq_sb[:, ec, :],
                        start=(ec == 0),
                        stop=(ec == EC - 1),
                    )
                # p = exp(scores + maskbias); maskbias = 0 kept, -1e4 masked
                nc.scalar.activation(
                    p_b[:, tt, :], sc_ps[:], EXP, bias=mb_t[:, t_ : t_ + 1]
                )
            uz_queue.append((b_, t0_, h_se_b, p_b))
            if len(uz_queue) > 1:
                emit_uz(uz_queue.pop(0))

        pending = []
        for ci, (b, s0, sl) in enumerate(chunks):
            x_sb = bufq.pop(0)
            for j in range(sl // (TB * P)):
                # --- projection for TB s-tiles ---
                h_se_b = hp.tile([P, TB, E], bf16, tag="h_se")
                for tt in range(TB):
                    ts = (j * TB + tt) * P
                    h_ps = ps_h.tile([P, E], f32)
                    for c in range(C4):
                        nc.tensor.matmul(
                            h_ps[:],
                            x_sb[:, c, :, ts : ts + P],
                            wp_sb[:, c, :, :],
                            start=(c == 0),
                            stop=(c == C4 - 1) and not has_bp,
                            perf_mode=DR,
                        )
                    if has_bp:
                        nc.tensor.matmul(
                            h_ps[:], ones_row[:], bp_sb[:], start=False, stop=True
                        )
                    # relu + cast on the vector engine (which runs ONLY
                    # these ops, so transposes never wait on anything else)
                    nc.vector.tensor_scalar_max(h_se_b[:, tt, :], h_ps[:], 0.0)
                # --- one batched SBUF->SBUF XBAR transpose for TB tiles.
                # Sync queue only: concurrent transposes from both HWDGE
                # queues race on the shared XBAR and corrupt data
                # (observed as nondeterministic output error) ---
                ht_b = htp.tile([P, TB * EC, P], bf16, tag="ht")
                nc.sync.dma_start_transpose(ht_b[:], h_se_b[:])
                pending.append((b, (s0 // P) + j * TB, h_se_b, ht_b))
                if len(pending) > 3:
                    emit_tails(pending.pop(0))
            while next_load < len(chunks) and next_load <= ci + 3:
                bufq.append(load_chunk(next_load))
                next_load += 1
            if s0 == 0 and b + 2 < BL:
                load_mb(b + 2)
        while pending:
            emit_tails(pending.pop(0))
        while uz_queue:
            emit_uz(uz_queue.pop(0))
        emit_drains()

    nc.compile()
    with _nc_lock:
        _nc_cache[key] = nc
    return nc


def prepare_core_inputs(x, mask, Wp8, wkq_b16, bp=None):
    """Host-side packing for ONE core's shard."""
    import ml_dtypes

    fp8 = ml_dtypes.float8_e4m3
    BL_, S_, I_ = x.shape
    C4 = I_ // 256
    # xt[b, ki, c, ko, s] = x[b, s, 256c + 128ko + ki]
    x8 = x.astype(fp8)
    xt = np.ascontiguousarray(
        x8.reshape(BL_, S_, C4, 2, P).transpose(0, 4, 2, 3, 1)
    )
    # additive mask bias packed [BL, P, S_TILES]: 0 where kept, -1e4 where
    # masked (exp(-1e4 + s) underflows to exactly 0)
    mb = np.ascontiguousarray(
        ((mask.astype(np.float32) - 1.0) * 1.0e4)
        .reshape(BL_, S_ // P, P)
        .transpose(0, 2, 1)
    ).astype(np.float32)
    m = {"xt": xt, "wp": Wp8, "wkq": wkq_b16, "mb": mb}
    if bp is not None:
        import ml_dtypes as md

        m["bp"] = (np.asarray(bp) * WP_SCALE).astype(md.bfloat16).reshape(1, E)
    return m


def kernel(
    x, mask, query, Wp, bp, Wq, bq, Wk, bk, Wv, bv, Wo, bo, W2, b2, gamma, beta,
    _trace=False,
):
    import ml_dtypes

    x = np.asarray(x)
    mask = np.asarray(mask)
    BL = B // NCORES

    # Host-side folds (all tiny)
    qh = (np.asarray(query, np.float64) @ np.asarray(Wq, np.float64)
          + np.asarray(bq, np.float64)).reshape(H, D)
    wkq_scaled = np.einsum(
        "ehd,hd->eh",
        np.asarray(Wk, np.float64).reshape(E, H, D),
        qh,
    ) / (np.sqrt(D) * WP_SCALE)
    EC = E // P
    wkq_b16 = np.ascontiguousarray(
        wkq_scaled.astype(np.float32).reshape(EC, P, H)
    ).astype(ml_dtypes.bfloat16)
    C4 = IN_DIM // 256
    Wp8 = np.ascontiguousarray(
        (np.asarray(Wp, np.float32) * WP_SCALE)
        .reshape(C4, 2, P, E)
        .transpose(2, 0, 1, 3)
    ).astype(ml_dtypes.float8_e4m3)

    has_bp = bool(np.any(np.asarray(bp)))
    nc = build_nc(has_bp=has_bp)

    in_maps = []
    for c in range(NCORES):
        sl = slice(c * BL, (c + 1) * BL)
        in_maps.append(
            prepare_core_inputs(
                x[sl], mask[sl], Wp8, wkq_b16,
                bp=np.asarray(bp) if has_bp else None,
            )
        )

    res = run_bass_kernel_spmd(
        nc, in_maps, core_ids=list(range(NCORES)), trace=_trace
    )
    u_raw = np.concatenate([r["u_out"] for r in res.results], axis=0)  # (B, P, E)
    z_raw = np.concatenate([r["z_out"] for r in res.results], axis=0)  # (B, TB*H, 1)
    # sum the 4 col-tile / head-group partials (garbage partitions ignored)
    U = sum(u_raw[:, 32 * j : 32 * j + H, :].astype(np.float64) for j in range(TB))
    Z = z_raw.astype(np.float64).reshape(B, TB, H, 1).sum(axis=1)  # (B, H, 1)

    # Host epilogue in float64
    pooledH = U / (Z * WP_SCALE)  # (B, H, E)
    Wv64 = np.asarray(Wv, np.float64).reshape(E, H, D)
    pooled = np.einsum("bhe,ehd->bhd", pooledH, Wv64).reshape(B, E)
    pooled += np.asarray(bv, np.float64)
    pooled = pooled @ np.asarray(Wo, np.float64) + np.asarray(bo, np.float64)
    out = pooled @ np.asarray(W2, np.float64) + np.asarray(b2, np.float64)
    mu = out.mean(-1, keepdims=True)
    var = out.var(-1, keepdims=True)
    out = (out - mu) / np.sqrt(var + 1e-5) * np.asarray(gamma, np.float64) + np.asarray(
        beta, np.float64
    )
    out_f32 = out.astype(np.float32)
    if _trace:
        return out_f32, res
    return out_f32


# revision 60
# speedup vs baseline: 1.0681x; 1.0681x over previous
"""Trainium2 Bass kernel for nn_AttentionPoolingTemporalEncoder.

Strategy (data-parallel over batch, 8 cores, 4 batch rows each):
  device:  h' = relu(x @ (64*Wp))          (fp8 DoubleRow matmuls, 2x rate)
           scores = h' @ ((Wk @ qh)/(64*sqrt(D)))  (bk shifts cancel in softmax)
           p = exp(scores + maskbias)      (no running max; scores are O(5))
           U[h,:] = sum_s p[s,h] * h'[s,:] ; Z[h] = sum_s p[s,h]
  host:    pooled = (U/(64*Z)) @ Wv (+bv) per head; @Wo+bo; @W2+b2; LayerNorm.

v5: the U matmuls (M=8, N=512) are col-tiled — the 4 tiles of each
transpose batch run concurrently in distinct 32-column PE groups
(tile_position=(0,32*tt)), quadrupling U throughput; the host sums the
4 partition groups. The vector engine runs ONLY the relu/cast ops:
every transpose waits on the single per-engine semaphore of its
producers, so any other op in that queue (exp, copies) transitively
couples the transpose to tensor-engine tail progress and stalls the
pipeline — exp and the U/Z drains therefore live on the scalar
engine, DMAs and memsets on gpsimd. x is loaded in 1MB chunks (small
first chunks on the HWDGE scalar queue before any compute queues up);
attention tails trail the projection by 3 transpose batches.
"""

import sys
import threading

import numpy as np

sys.path.insert(0, "/opt/trn_rl_repo")

from contextlib import ExitStack

import concourse.tile as tile
from concourse import bacc, mybir
from concourse.bass_utils import run_bass_kernel_spmd


def _ensure_axon_ntff_hook_module():
    """Some images lack ``antenv.axon_hooks``; concourse imports it
    unconditionally when tracing is requested (e.g. via BASS_TRACE).
    Provide a minimal stand-in so that path degrades to no-trace
    instead of crashing."""
    try:
        from antenv import axon_hooks  # noqa: F401

        return
    except ImportError:
        pass
    import types

    mod = types.ModuleType("antenv.axon_hooks")
    mod._hook = None

    def set_axon_ntff_profile_hook(h):
        mod._hook = h

    def get_axon_ntff_profile_hook():
        return mod._hook

    mod.set_axon_ntff_profile_hook = set_axon_ntff_profile_hook
    mod.get_axon_ntff_profile_hook = get_axon_ntff_profile_hook
    sys.modules["antenv.axon_hooks"] = mod
    try:
        import antenv

        antenv.axon_hooks = mod
    except ImportError:
        pass


_ensure_axon_ntff_hook_module()

# Problem sizes (hardcoded per spec)
B, S, IN_DIM, E, H = 32, 4096, 1024, 512, 8
D = E // H
NCORES = 8
P = 128
WP_SCALE = 64.0  # Wp pre-scaled into fp8's sweet spot; relu is homogeneous
TB = 4           # s-tiles per transpose batch / U col-tile group

_nc_cache = {}
_nc_lock = threading.Lock()


def build_nc(BL=B // NCORES, S_=S, I_=IN_DIM, has_bp=False):
    """Build + compile the per-core Bass program."""
    key = (BL, S_, I_, has_bp)
    with _nc_lock:
        if key in _nc_cache:
            return _nc_cache[key]

    C4 = I_ // 256      # 256-deep DoubleRow contraction chunks
    EC = E // P         # embed-dim chunks
    S_TILES = S_ // P   # s-tiles per batch row

    # x chunk list: (row, s_start, s_len). Small first chunks get the
    # pipeline started early; 1024-s (1MB) chunks keep SDMA ring backlog
    # short so transposes are never stuck behind a large x transfer.
    chunks = []
    for bb in range(BL):
        s0 = 0
        first = bb == 0
        while s0 < S_:
            sl = 512 if (first and s0 < 1024) else 1024
            sl = min(sl, S_ - s0)
            chunks.append((bb, s0, sl))
            s0 += sl

    f32 = mybir.dt.float32
    bf16 = mybir.dt.bfloat16
    fp8 = mybir.dt.float8e4
    EXP = mybir.ActivationFunctionType.Exp
    RELU = mybir.ActivationFunctionType.Relu
    DR = mybir.MatmulPerfMode.DoubleRow

    nc = bacc.Bacc(
        "TRN2",
        target_bir_lowering=False,
        debug=False,
        enable_asserts=False,
        num_devices=NCORES,
    )

    xt = nc.dram_tensor("xt", [BL, P, C4, 2, S_], fp8, kind="ExternalInput").ap()
    wp = nc.dram_tensor("wp", [P, C4, 2, E], fp8, kind="ExternalInput").ap()
    wkq = nc.dram_tensor("wkq", [EC, P, H], bf16, kind="ExternalInput").ap()
    mb = nc.dram_tensor("mb", [BL, P, S_TILES], f32, kind="ExternalInput").ap()
    if has_bp:
        bp_d = nc.dram_tensor("bp", [1, E], bf16, kind="ExternalInput").ap()
    # col-tiled U: partition group 32*j holds the partial sums of
    # s-tiles with (tile_in_batch == j); host adds the 4 groups.
    u_out = nc.dram_tensor("u_out", [BL, P, E], f32, kind="ExternalOutput").ap()
    z_out = nc.dram_tensor("z_out", [BL, H, 1], f32, kind="ExternalOutput").ap()

    with tile.TileContext(nc) as tc, ExitStack() as ctx:
        const = ctx.enter_context(tc.tile_pool(name="const", bufs=1))
        xp = ctx.enter_context(tc.tile_pool(name="xp", bufs=6))
        hp = ctx.enter_context(tc.tile_pool(name="hp", bufs=7))
        h8p = ctx.enter_context(tc.tile_pool(name="h8p", bufs=7))
        htp = ctx.enter_context(tc.tile_pool(name="htp", bufs=7))
        pp = ctx.enter_context(tc.tile_pool(name="pp", bufs=6))
        mbp = ctx.enter_context(tc.tile_pool(name="mbp", bufs=2))
        uzp = ctx.enter_context(tc.tile_pool(name="uzp", bufs=2))
        ps_h = ctx.enter_context(tc.tile_pool(name="ps_h", bufs=3, space="PSUM"))
        ps_s = ctx.enter_context(tc.tile_pool(name="ps_s", bufs=2, space="PSUM"))
        ps_u = ctx.enter_context(tc.tile_pool(name="ps_u", bufs=2, space="PSUM"))
        ps_z = ctx.enter_context(tc.tile_pool(name="ps_z", bufs=1, space="PSUM"))

        # Resident constants (sync queue, ahead of the transposes)
        wp_sb = const.tile([P, C4, 2, E], fp8)
        nc.sync.dma_start(wp_sb[:], wp[:])
        wkq_sb = const.tile([P, EC, H], bf16)
        nc.sync.dma_start(wkq_sb[:], wkq.rearrange("c p h -> p c h"))
        ones_t = const.tile([P, 2, 16], fp8)
        nc.gpsimd.memset(ones_t[:], 1.0)
        if has_bp:
            ones_row = const.tile([1, P], bf16)
            nc.gpsimd.memset(ones_row[:], 1.0)
            bp_sb = const.tile([1, E], bf16)
            nc.sync.dma_start(bp_sb[:], bp_d[:])

        # x chunks on the scalar HWDGE queue. The issue distance (3) is
        # kept well below the buffer count (6) so each dma_start's
        # buffer-reuse wait refers to a chunk finished long ago — the DMA
        # fires the moment the queue reaches it and the chunk lands well
        # before the sync queue hits Tile's completion-observer for it
        # (those observers are sequenced between the transposes).
        def load_chunk(idx):
            bb, s0, sl = chunks[idx]
            xt_c = xp.tile([P, C4, 2, sl], fp8, tag="xchunk")
            # scalar HWDGE: the queue's natural (exp-paced) progress rate
            # spreads the x transfers out so they never flood the shared
            # SDMA rings and starve the transposes
            nc.scalar.dma_start(xt_c[:], xt[bb, :, :, :, s0 : s0 + sl])
            return xt_c

        # shallow initial prefetch (ring-flood at kernel start delays the
        # first transposes' completion-observers); ramps to distance 3
        bufq = [load_chunk(i) for i in range(min(2, len(chunks)))]
        next_load = len(bufq)

        mb_tiles = {}

        def load_mb(bb):
            mb_t = mbp.tile([P, S_TILES], f32)
            nc.gpsimd.dma_start(mb_t[:], mb[bb])
            mb_tiles[bb] = mb_t

        load_mb(0)
        if BL > 1:
            load_mb(1)

        row_state = {}  # b -> (u_ps, z_ps)
        COPY = mybir.ActivationFunctionType.Copy
        uz_queue = []     # stage-B work: (b, t0, h_se_b, p_b)
        drain_queue = []  # row drains, delayed >= 1 batch so the scalar
                          # copies never wait on in-flight U matmuls

        def emit_drains():
            while drain_queue:
                b_, u_ps, z_ps = drain_queue.pop(0)
                u_sb = uzp.tile([P, E], f32, tag="u_sb")
                z_sb = uzp.tile([H, 1], f32, tag="z_sb")
                nc.scalar.activation(u_sb[:], u_ps[:], COPY)
                nc.scalar.activation(z_sb[:], z_ps[:, 0:1], COPY)
                nc.scalar.dma_start(u_out[b_], u_sb[:])
                nc.scalar.dma_start(z_out[b_], z_sb[:])

        def emit_uz(work):
            """Stage B: U/Z matmuls, one transpose batch behind stage A
            so the exps have a full batch period to complete."""
            b_, t0_, h8_b, p_b = work
            u_ps, z_ps = row_state[b_]
            # U as 2 fp8 DoubleRow matmuls (adjacent s-tiles paired into
            # the K dimension), both accumulating into the same region
            # (DR forbids non-zero destination col groups)
            for j in range(TB // 2):
                nc.tensor.matmul(
                    u_ps[0:H, :],
                    p_b[:, 2 * j : 2 * j + 2, 0:H],
                    h8_b[:, 2 * j : 2 * j + 2, :],
                    start=(t0_ == 0 and j == 0),
                    stop=(t0_ + TB == S_TILES and j == TB // 2 - 1),
                    skip_group_check=True,
                    perf_mode=DR,
                )
            # Z as 2 fp8 DoubleRow matmuls over the same p pairs
            for j in range(TB // 2):
                nc.tensor.matmul(
                    z_ps[:],
                    p_b[:, 2 * j : 2 * j + 2, 0:H],
                    ones_t[:, :, 0:2],
                    start=(t0_ == 0 and j == 0),
                    stop=(t0_ + TB == S_TILES and j == TB // 2 - 1),
                    skip_group_check=True,
                    perf_mode=DR,
                )
            if t0_ + TB == S_TILES:
                drain_queue.append((b_, u_ps, z_ps))
                del row_state[b_]

        def emit_tails(pend):
            """Stage A for one transpose batch: scores + exp; then stage
            B (U/Z) for the previous batch and any due row drains."""
            b_, t0_, h8_b, ht_b = pend
            emit_drains()
            if b_ not in row_state:
                u_ps = ps_u.tile([P, E], f32)
                z_ps = ps_z.tile([H, 2], f32)
                row_state[b_] = (u_ps, z_ps)
            mb_t = mb_tiles[b_]
            # p padded to 16 cols: the DoubleRow weights AP needs a
            # 16-byte-aligned stride on the pair axis
            p_b = pp.tile([P, TB, 16], fp8)
            for tt in range(TB):
                t_ = t0_ + tt
                # scores[s,h] = sum_e h[s,e] wkq[e,h]
                sc_ps = ps_s.tile([P, H], f32)
                for ec in range(EC):
                    nc.tensor.matmul(
                        sc_ps[:],
                        ht_b[:, tt * EC + ec, :],
                        wkq_sb[:, ec, :],
                        start=(ec == 0),
                        stop=(ec == EC - 1),
                    )
                # p = exp(scores + maskbias)/16; maskbias = -ln16 kept,
                # -1e4 masked (the 1/16 keeps fp8 p well inside e4m3
                # range; U/Z is scale-invariant so the host is unchanged)
                nc.scalar.activation(
                    p_b[:, tt, 0:H], sc_ps[:], EXP, bias=mb_t[:, t_ : t_ + 1]
                )
            uz_queue.append((b_, t0_, h8_b, p_b))
            if len(uz_queue) > 1:
                emit_uz(uz_queue.pop(0))

        pending = []
        for ci, (b, s0, sl) in enumerate(chunks):
            x_sb = bufq.pop(0)
            for j in range(sl // (TB * P)):
                # --- projection for TB s-tiles ---
                h_se_b = hp.tile([P, TB, E], bf16, tag="h_se")
                h8_b = h8p.tile([P, TB, E], fp8, tag="h8")
                for tt in range(TB):
                    ts = (j * TB + tt) * P
                    h_ps = ps_h.tile([P, E], f32)
                    for c in range(C4):
                        nc.tensor.matmul(
                            h_ps[:],
                            x_sb[:, c, :, ts : ts + P],
                            wp_sb[:, c, :, :],
                            start=(c == 0),
                            stop=(c == C4 - 1) and not has_bp,
                            perf_mode=DR,
                        )
                    if has_bp:
                        nc.tensor.matmul(
                            h_ps[:], ones_row[:], bp_sb[:], start=False, stop=True
                        )
                    # relu + cast on the vector engine (which runs ONLY
                    # these ops, so transposes never wait on anything else);
                    # a second fp8 copy for the DoubleRow U matmuls goes on
                    # the scalar engine
                    nc.vector.tensor_scalar_max(h_se_b[:, tt, :], h_ps[:], 0.0)
                    # 1/16 pre-scale keeps the fp8 copy inside e4m3 range
                    # (h is ~N(0,64^2); unscaled tails overflow 240 -> inf)
                    nc.scalar.activation(
                        h8_b[:, tt, :], h_ps[:], RELU, scale=1.0 / 16.0
                    )
                # --- one batched SBUF->SBUF XBAR transpose for TB tiles.
                # Sync queue only: concurrent transposes from both HWDGE
                # queues race on the shared XBAR and corrupt data
                # (observed as nondeterministic output error) ---
                ht_b = htp.tile([P, TB * EC, P], bf16, tag="ht")
                nc.sync.dma_start_transpose(ht_b[:], h_se_b[:])
                pending.append((b, (s0 // P) + j * TB, h8_b, ht_b))
                if len(pending) > 3:
                    emit_tails(pending.pop(0))
            while next_load < len(chunks) and next_load <= ci + 3:
                bufq.append(load_chunk(next_load))
                next_load += 1
            if s0 == 0 and b + 2 < BL:
                load_mb(b + 2)
        while pending:
            emit_tails(pending.pop(0))
        while uz_queue:
            emit_uz(uz_queue.pop(0))
        emit_drains()

    nc.compile()
    with _nc_lock:
        _nc_cache[key] = nc
    return nc


def prepare_core_inputs(x, mask, Wp8, wkq_b16, bp=None):
    """Host-side packing for ONE core's shard."""
    import ml_dtypes

    fp8 = ml_dtypes.float8_e4m3
    BL_, S_, I_ = x.shape
    C4 = I_ // 256
    # xt[b, ki, c, ko, s] = x[b, s, 256c + 128ko + ki]
    x8 = x.astype(fp8)
    xt = np.ascontiguousarray(
        x8.reshape(BL_, S_, C4, 2, P).transpose(0, 4, 2, 3, 1)
    )
    # additive mask bias packed [BL, P, S_TILES]: -ln16 where kept (keeps
    # fp8 p inside e4m3 range; U/Z is invariant to the scale), -1e4 where
    # masked (exp underflows to exactly 0)
    mb = np.ascontiguousarray(
        ((mask.astype(np.float32) - 1.0) * 1.0e4 - np.log(16.0))
        .reshape(BL_, S_ // P, P)
        .transpose(0, 2, 1)
    ).astype(np.float32)
    m = {"xt": xt, "wp": Wp8, "wkq": wkq_b16, "mb": mb}
    if bp is not None:
        import ml_dtypes as md

        m["bp"] = (np.asarray(bp) * WP_SCALE).astype(md.bfloat16).reshape(1, E)
    return m


def kernel(
    x, mask, query, Wp, bp, Wq, bq, Wk, bk, Wv, bv, Wo, bo, W2, b2, gamma, beta,
    _trace=False,
):
    import ml_dtypes

    x = np.asarray(x)
    mask = np.asarray(mask)
    BL = B // NCORES

    # Host-side folds (all tiny)
    qh = (np.asarray(query, np.float64) @ np.asarray(Wq, np.float64)
          + np.asarray(bq, np.float64)).reshape(H, D)
    wkq_scaled = np.einsum(
        "ehd,hd->eh",
        np.asarray(Wk, np.float64).reshape(E, H, D),
        qh,
    ) / (np.sqrt(D) * WP_SCALE)
    EC = E // P
    wkq_b16 = np.ascontiguousarray(
        wkq_scaled.astype(np.float32).reshape(EC, P, H)
    ).astype(ml_dtypes.bfloat16)
    C4 = IN_DIM // 256
    Wp8 = np.ascontiguousarray(
        (np.asarray(Wp, np.float32) * WP_SCALE)
        .reshape(C4, 2, P, E)
        .transpose(2, 0, 1, 3)
    ).astype(ml_dtypes.float8_e4m3)

    has_bp = bool(np.any(np.asarray(bp)))
    nc = build_nc(has_bp=has_bp)

    in_maps = []
    for c in range(NCORES):
        sl = slice(c * BL, (c + 1) * BL)
        in_maps.append(
            prepare_core_inputs(
                x[sl], mask[sl], Wp8, wkq_b16,
                bp=np.asarray(bp) if has_bp else None,
            )
        )

    res = run_bass_kernel_spmd(
        nc, in_maps, core_ids=list(range(NCORES)), trace=_trace
    )
    u_raw = np.concatenate([r["u_out"] for r in res.results], axis=0)  # (B, P, E)
    z_raw = np.concatenate([r["z_out"] for r in res.results], axis=0)  # (B, H, 1)
    U = u_raw[:, 0:H, :].astype(np.float64)  # (B, H, E); rest is garbage
    Z = z_raw.astype(np.float64)  # (B, H, 1)

    # Host epilogue in float64 (the fp8 U path carries h at 1/16 scale)
    pooledH = U / (Z * (WP_SCALE / 16.0))  # (B, H, E)
    Wv64 = np.asarray(Wv, np.float64).reshape(E, H, D)
    pooled = np.einsum("bhe,ehd->bhd", pooledH, Wv64).reshape(B, E)
    pooled += np.asarray(bv, np.float64)
    pooled = pooled @ np.asarray(Wo, np.float64) + np.asarray(bo, np.float64)
    out = pooled @ np.asarray(W2, np.float64) + np.asarray(b2, np.float64)
    mu = out.mean(-1, keepdims=True)
    var = out.var(-1, keepdims=True)
    out = (out - mu) / np.sqrt(var + 1e-5) * np.asarray(gamma, np.float64) + np.asarray(
        beta, np.float64
    )
    out_f32 = out.astype(np.float32)
    if _trace:
        return out_f32, res
    return out_f32
